# revision 1
# baseline (speedup 1.0000x reference)
"""Trainium2 Bass kernel for nn_BatchedChebLayer (gnn_message_passing).

Strategy (8 NeuronCores, SPMD):
- Flatten features: h = x.transpose(1,0,2).reshape(N, T*C) = [50000, 512],
  cast bf16. Chebyshev: out = x@W0' + S1@W1 + S2@W2' + bias with
  S1 = L@h, S2 = L@S1, W0' = W0-W2, W2' = 2*W2 (host-folded).
- Core c owns 6250 output rows dealt degree-balanced into 49 windows of
  128. Each SPMM hop: edges packed into 128-token blocks per (window,
  table-half); blocks gathered 8-at-a-time with gpsimd.dma_gather (int16
  idxs, bf16 rows, 4 SWDGE queues round-robin) -- ~5x faster than
  per-block indirect DMA which is SWDGE-emission-bound (~1us fixed/call).
  Table split at row 32768 into two DRAM tensors so indices fit int16.
- Selection matrices (scatter-free accumulation) are built ON DEVICE:
  sel[p,m] = (iota[m] == lrow[p]) * val[p] via one fused DVE
  tensor_scalar(is_equal, mult) per block; psum accumulates f32; output
  cast bf16 (feeds hop2 table + dense stage).
- Dense stage: big contiguous bf16 loads [128, 6656] per (t,k), 13 psum
  tiles x 3 matmuls, outT bf16.
- Host between launches: assemble S1/S2 tables from per-core outputs
  (full_io contract; device time is what counts).
"""
import sys
sys.path.insert(0, "/opt/trn_rl_repo")
sys.path.insert(0, "/root/.axon_site/_ro/trn_rl_repo")
import numpy as np
import ml_dtypes

T, N, E, C, KCH = 4, 50000, 800000, 128, 3
D = T * C                  # 512 flat features
NCORES = 8
RPC = N // NCORES          # 6250 rows per core
PTILES = (RPC + 127) // 128            # 49 psum tiles (windows) per core
RPAD = PTILES * 128                    # 6272 padded rows per core
SPLIT = 32768                          # table halves: [0,SPLIT), [SPLIT,N)
NHI = N - SPLIT                        # 17232
QNQ = 4
GB = 4                                 # blocks per dma_gather call
NIPC = GB * 128
CPW = NIPC // 16                       # idx cols per call
SP = True                              # dma_gather single_packet
GBUFS = 16
SELMODE = "dve"                        # "dve" or "dma"
PSBUFS = 6
SELBUFS = 6


def QN(i):
    return i % QNQ


def QMAP(s, i):
    if QSPLIT is None:
        return i % QNQ
    qs = QSPLIT[s]
    return qs[i % len(qs)]


QSPLIT = None                          # e.g. ([0,1,2],[3]) = per-stream queues
DW = 512                               # dense psum width
DTILES = 13
RPAD_D = DTILES * DW                   # 6656

_cache = {}
BF16 = ml_dtypes.bfloat16


def build_schedule(edge_row, edge_col, edge_val):
    """Degree-balanced deal + per-(window, table-half) block packing.

    Returns dict with:
      row_of   [NCORES, RPAD] global row id per (core, local row), -1 pad
      blocks   list per tile w: (BLO, BHI)
      tilemeta list per tile w: list of (stream, call, kk) per block
      ncalls   (ncalls_lo, ncalls_hi)
      idx_lo/hi [NCORES, 128, ncalls*64] int16
      lrow     [NCORES, 128, NB] f32   (target row within window, 0..127)
      val      [NCORES, 128, NB] f32   (edge weight; 0 for pad slots)
      NB       total blocks per tile-sweep (one hop)
    """
    order = np.argsort(edge_row, kind="stable")
    rows = edge_row[order].astype(np.int64)
    cols = edge_col[order].astype(np.int64)
    vals = edge_val[order].astype(np.float32)

    nwin = PTILES
    deg = np.bincount(edge_row, minlength=N)
    srows = np.argsort(-deg, kind="stable")
    nbuck = NCORES * nwin
    pos = np.arange(N)
    p, j = pos // nbuck, pos % nbuck
    buck = np.where(p % 2 == 0, j, nbuck - 1 - j)
    core_of = np.empty(N, np.int64)
    lrow_of = np.empty(N, np.int64)
    core_of[srows] = buck % NCORES
    win_of = buck // NCORES
    occ = np.zeros(N, np.int64)
    sort_b = np.argsort(buck, kind="stable")
    bs = buck[sort_b]
    starts = np.searchsorted(bs, np.arange(nbuck), side="left")
    occ[sort_b] = np.arange(N) - starts[bs]
    assert occ.max() < 128, "bucket overflow"
    lrow_of[srows] = win_of * 128 + occ
    row_of = np.full((NCORES, RPAD), -1, np.int64)
    row_of[core_of, lrow_of] = np.arange(N)

    # per-token: core, window, local row m, stream, local col idx
    tcore = core_of[rows]
    tlrow = lrow_of[rows]
    twin = tlrow // 128
    tm = tlrow % 128
    tstream = (cols >= SPLIT).astype(np.int64)
    tjloc = np.where(tstream == 0, cols, cols - SPLIT)

    # counts per (core, window, stream)
    key = (tcore * nwin + twin) * 2 + tstream
    cnt = np.bincount(key, minlength=NCORES * nwin * 2).reshape(
        NCORES, nwin, 2)
    bw = -(-cnt.max(axis=0) // 128)          # [nwin, 2] blocks per stream
    BLO, BHI = bw[:, 0], bw[:, 1]
    NB = int(bw.sum())

    # tile-local block ordinals and global stream block indices
    lo_base = np.zeros(nwin + 1, np.int64)
    np.cumsum(BLO, out=lo_base[1:])
    hi_base = np.zeros(nwin + 1, np.int64)
    np.cumsum(BHI, out=hi_base[1:])
    tile_base = np.zeros(nwin + 1, np.int64)
    np.cumsum(BLO + BHI, out=tile_base[1:])
    ncalls_lo = int(-(-lo_base[-1] // GB))
    ncalls_hi = int(-(-hi_base[-1] // GB))

    tilemeta = []
    for w in range(nwin):
        meta = []
        for k in range(int(BLO[w])):
            sbi = lo_base[w] + k
            meta.append((0, int(sbi // GB), int(sbi % GB)))
        for k in range(int(BHI[w])):
            sbi = hi_base[w] + k
            meta.append((1, int(sbi // GB), int(sbi % GB)))
        tilemeta.append(meta)

    idx_lo = np.zeros((NCORES, 16, ncalls_lo * CPW), np.int16)
    idx_hi = np.zeros((NCORES, 16, ncalls_hi * CPW), np.int16)
    lrow = np.zeros((NCORES, 128, NB), np.float32)
    val = np.zeros((NCORES, 128, NB), np.float32)

    # slot assignment: stable-sort tokens by (core, window, stream)
    sort2 = np.argsort(key, kind="stable")
    ks = key[sort2]
    gstarts = np.searchsorted(ks, np.arange(NCORES * nwin * 2), side="left")
    q = np.arange(E) - gstarts[ks]           # slot within (c,w,s) group
    c2 = tcore[sort2]
    w2 = twin[sort2]
    s2 = tstream[sort2]
    m2 = tm[sort2]
    j2 = tjloc[sort2]
    v2 = vals[sort2]
    kblk = q // 128                          # block within (w, s)
    pslot = q % 128
    # tile-local block ordinal
    bo = np.where(s2 == 0, kblk, BLO[w2] + kblk) + tile_base[w2]
    lrow[c2, pslot, bo] = m2
    val[c2, pslot, bo] = v2
    # gather stream position
    sbi = np.where(s2 == 0, lo_base[w2], hi_base[w2]) + kblk
    cidx = sbi // GB
    kk = sbi % GB
    gi = kk * 128 + pslot                    # index within call
    gcol = cidx * CPW + gi // 16
    gpart = gi % 16
    idx16 = j2.astype(np.int16)
    m_lo = s2 == 0
    idx_lo[c2[m_lo], gpart[m_lo], gcol[m_lo]] = idx16[m_lo]
    m_hi = ~m_lo
    idx_hi[c2[m_hi], gpart[m_hi], gcol[m_hi]] = idx16[m_hi]
    # replicate the 16-partition index groups x8 (one copy per Q7 core)
    idx_lo = np.ascontiguousarray(np.tile(idx_lo, (1, 8, 1)))
    idx_hi = np.ascontiguousarray(np.tile(idx_hi, (1, 8, 1)))

    assert max(len(m) for m in tilemeta) <= 24, "sel tile too small"
    assert min(len(m) for m in tilemeta) >= 1
    blocks = [(int(BLO[w]), int(BHI[w])) for w in range(nwin)]
    return dict(row_of=row_of, blocks=blocks, tilemeta=tilemeta,
                ncalls=(ncalls_lo, ncalls_hi), idx_lo=idx_lo, idx_hi=idx_hi,
                lrow=lrow, val=val, NB=NB)


def build_spmm(sched, repeat=1):
    import concourse.bacc as bacc
    import concourse.tile as tile
    import concourse.mybir as mybir
    from concourse import library_config

    ncalls_lo, ncalls_hi = sched["ncalls"]
    NBt = sched["NB"]
    tilemeta = sched["tilemeta"]
    selmax = max(len(m) for m in tilemeta)
    nc = bacc.Bacc("TRN2", target_bir_lowering=False, debug=False,
                   num_devices=NCORES, num_swdge_queues=4)
    bf = mybir.dt.bfloat16
    tlo = nc.dram_tensor("tlo", [SPLIT, D], bf, kind="ExternalInput")
    thi = nc.dram_tensor("thi", [NHI, D], bf, kind="ExternalInput")
    idxlo = nc.dram_tensor("idxlo", [128, ncalls_lo * CPW], mybir.dt.int16,
                           kind="ExternalInput")
    idxhi = nc.dram_tensor("idxhi", [128, ncalls_hi * CPW], mybir.dt.int16,
                           kind="ExternalInput")
    lrow = nc.dram_tensor("lrow", [128, NBt], mybir.dt.float32,
                          kind="ExternalInput")
    val = nc.dram_tensor("val", [128, NBt], mybir.dt.float32,
                         kind="ExternalInput")
    iota = nc.dram_tensor("iota", [128, 128], mybir.dt.float32,
                          kind="ExternalInput")
    if SELMODE == "dma":
        seldram = nc.dram_tensor("sel", [128, NBt * 128], bf,
                                 kind="ExternalInput")
    sout = nc.dram_tensor("sout", [RPAD, D], bf, kind="ExternalOutput")
    tabs = [tlo, thi]
    idxts = []
    with tile.TileContext(nc) as tc:
        with (
            tc.tile_pool(name="const", bufs=1) as cpool,
            tc.tile_pool(name="glo", bufs=GBUFS) as glopool,
            tc.tile_pool(name="ghi", bufs=GBUFS) as ghipool,
            tc.tile_pool(name="selp", bufs=SELBUFS) as selpool,
            tc.tile_pool(name="stp", bufs=3) as stpool,
            tc.tile_pool(name="psum", bufs=PSBUFS, space="PSUM") as ppool,
        ):
            nc.gpsimd.load_library(library_config.mlp)
            it_lo = cpool.tile([128, ncalls_lo * CPW], mybir.dt.int16)
            nc.sync.dma_start(it_lo[:], idxlo[:])
            it_hi = cpool.tile([128, ncalls_hi * CPW], mybir.dt.int16)
            nc.sync.dma_start(it_hi[:], idxhi[:])
            lrow_t = cpool.tile([128, NBt], mybir.dt.float32)
            nc.sync.dma_start(lrow_t[:], lrow[:])
            val_t = cpool.tile([128, NBt], mybir.dt.float32)
            nc.sync.dma_start(val_t[:], val[:])
            iota_t = cpool.tile([128, 128], mybir.dt.float32)
            nc.sync.dma_start(iota_t[:], iota[:])
            idxts = [it_lo, it_hi]
            gpools = [glopool, ghipool]
            total_calls = [ncalls_lo, ncalls_hi]
            nblocks_stream = [0, 0]
            for meta in tilemeta:
                for (s, cidx, kk) in meta:
                    nblocks_stream[s] = max(nblocks_stream[s], cidx * GB + kk + 1)
            with tc.For_i(0, repeat):
                gtiles = [{}, {}]
                qctr = [0]

                def issue(s, cidx):
                    nb_in_call = min(GB, nblocks_stream[s] - cidx * GB)
                    g = gpools[s].tile([128, GB * D], mybir.dt.bfloat16,
                                       tag=f"g{s}")
                    ni = nb_in_call * 128
                    gv = g[:, :nb_in_call * D].rearrange(
                        "p (k e) -> p k e", k=nb_in_call)
                    nc.gpsimd.dma_gather(
                        gv, tabs[s][:],
                        idxts[s][:, cidx * CPW:cidx * CPW + nb_in_call * 8],
                        ni, ni, D, queue_num=QMAP(s, qctr[0]),
                        single_packet=SP)
                    qctr[0] += 1
                    gtiles[s][cidx] = g

                bo = 0
                for w in range(PTILES):
                    meta = tilemeta[w]
                    nbw = len(meta)
                    sel_t = selpool.tile([128, selmax * 128],
                                         mybir.dt.bfloat16, tag="sel")
                    ps = ppool.tile([128, D], mybir.dt.float32)
                    if SELMODE == "dma":
                        nc.sync.dma_start(
                            sel_t[:, :nbw * 128],
                            seldram[:, bo * 128:(bo + nbw) * 128])
                    for bi, (s, cidx, kk) in enumerate(meta):
                        if cidx not in gtiles[s]:
                            issue(s, cidx)
                        if SELMODE == "dve":
                            nc.vector.tensor_scalar(
                                sel_t[:, bi * 128:(bi + 1) * 128],
                                iota_t[:],
                                lrow_t[:, bo + bi:bo + bi + 1],
                                val_t[:, bo + bi:bo + bi + 1],
                                mybir.AluOpType.is_equal,
                                mybir.AluOpType.mult)
                        nc.tensor.matmul(
                            out=ps[:],
                            lhsT=sel_t[:, bi * 128:(bi + 1) * 128],
                            rhs=gtiles[s][cidx][:, kk * D:(kk + 1) * D],
                            start=(bi == 0), stop=(bi == nbw - 1))
                    st = stpool.tile([128, D], mybir.dt.bfloat16, tag="st")
                    nc.scalar.copy(st[:], ps[:])
                    nc.sync.dma_start(sout[w * 128:(w + 1) * 128, :], st[:])
                    bo += nbw
    nc.compile()
    return nc


def build_dense(repeat=1):
    import concourse.bacc as bacc
    import concourse.tile as tile
    import concourse.mybir as mybir

    nc = bacc.Bacc("TRN2", target_bir_lowering=False, debug=False,
                   num_devices=NCORES)
    bf = mybir.dt.bfloat16
    xT = nc.dram_tensor("xT", [D, RPAD_D], bf, kind="ExternalInput")
    s1T = nc.dram_tensor("s1T", [D, RPAD_D], bf, kind="ExternalInput")
    s2T = nc.dram_tensor("s2T", [D, RPAD_D], bf, kind="ExternalInput")
    wp = nc.dram_tensor("wp", [C, T * KCH * C], bf, kind="ExternalInput")
    outT = nc.dram_tensor("outT", [T, C, RPAD_D], bf, kind="ExternalOutput")
    srcs = [xT, s1T, s2T]
    with tile.TileContext(nc) as tc:
        with (
            tc.tile_pool(name="wpool", bufs=1) as wpool,
            tc.tile_pool(name="rhsp", bufs=2) as rhspool,
            tc.tile_pool(name="outp", bufs=2) as outpool,
            tc.tile_pool(name="psum", bufs=4, space="PSUM") as ppool,
        ):
            w_t = wpool.tile([128, T * KCH * C], bf)
            nc.sync.dma_start(w_t[:], wp[:])
            with tc.For_i(0, repeat):
                for t in range(T):
                    rhss = []
                    for k in range(KCH):
                        rhs = rhspool.tile([128, RPAD_D], bf, tag=f"rhs{k}")
                        nc.sync.dma_start(rhs[:], srcs[k][t * C:(t + 1) * C, :])
                        rhss.append(rhs)
                    ot = outpool.tile([128, RPAD_D], bf, tag="ot")
                    for dw in range(DTILES):
                        ps = ppool.tile([128, DW], mybir.dt.float32)
                        for k in range(KCH):
                            nc.tensor.matmul(
                                out=ps[:],
                                lhsT=w_t[:, (t * KCH + k) * C:(t * KCH + k + 1) * C],
                                rhs=rhss[k][:, dw * DW:(dw + 1) * DW],
                                start=(k == 0), stop=(k == KCH - 1))
                        nc.vector.tensor_copy(ot[:, dw * DW:(dw + 1) * DW], ps[:])
                    nc.sync.dma_start(outT[t, :, :], ot[:])
    nc.compile()
    return nc


_IOTA = np.tile(np.arange(128, dtype=np.float32), (128, 1))


def host_sel(sched):
    if "hostsel" in sched:
        return sched["hostsel"]
    NB = sched["NB"]
    sel = np.zeros((NCORES, 128, NB, 128), BF16)
    li = sched["lrow"].astype(np.int64)
    cc, pp, bb = np.meshgrid(np.arange(NCORES), np.arange(128),
                             np.arange(NB), indexing="ij")
    sel[cc, pp, bb, li] = sched["val"].astype(BF16)
    sched["hostsel"] = sel.reshape(NCORES, 128, NB * 128)
    return sched["hostsel"]


def _spmm_inputs(sched, tab_bf):
    """Per-core input dicts for one hop given the bf16 table [N, D]."""
    tlo = np.ascontiguousarray(tab_bf[:SPLIT])
    thi = np.ascontiguousarray(tab_bf[SPLIT:])
    ins = [
        {"tlo": tlo, "thi": thi,
         "idxlo": sched["idx_lo"][c], "idxhi": sched["idx_hi"][c],
         "lrow": sched["lrow"][c], "val": sched["val"][c], "iota": _IOTA}
        for c in range(NCORES)]
    if SELMODE == "dma":
        hs = host_sel(sched)
        for c in range(NCORES):
            ins[c]["sel"] = hs[c]
    return ins


def kernel(x, edge_row, edge_col, edge_val, weight, bias):
    from concourse.bass_utils import run_bass_kernel_spmd

    x = np.asarray(x, dtype=np.float32)
    edge_row = np.asarray(edge_row).astype(np.int64)
    edge_col = np.asarray(edge_col).astype(np.int64)
    edge_val = np.asarray(edge_val, dtype=np.float32)
    weight = np.asarray(weight, dtype=np.float32)
    bias = np.asarray(bias, dtype=np.float32)

    h = np.ascontiguousarray(x.transpose(1, 0, 2).reshape(N, D))  # [N, T*C]
    h_bf = h.astype(BF16)
    wp = np.stack(
        [weight[:, 0] - weight[:, 2], weight[:, 1], 2.0 * weight[:, 2]],
        axis=1)  # [T, 3, C, C]
    wp = np.ascontiguousarray(
        wp.transpose(2, 0, 1, 3).reshape(C, T * KCH * C)).astype(BF16)

    fp = (E, int(edge_row[:64].sum()), int(edge_col[:64].sum()),
          int(edge_row[-64:].sum()), int(edge_col[-64:].sum()))
    if _cache.get("fp") != fp:
        _cache.clear()
        _cache["fp"] = fp
        _cache["sched"] = build_schedule(edge_row, edge_col, edge_val)
    sched = _cache["sched"]
    if "spmm" not in _cache:
        _cache["spmm"] = build_spmm(sched)
    if "dense" not in _cache:
        _cache["dense"] = build_dense()
    nc_s, nc_d = _cache["spmm"], _cache["dense"]
    row_of = sched["row_of"]
    valid = row_of >= 0
    clamped = np.maximum(row_of, 0)

    def run_hop(tab_bf):
        r = run_bass_kernel_spmd(nc_s, _spmm_inputs(sched, tab_bf),
                                 core_ids=list(range(NCORES)))
        return [r.results[c]["sout"] for c in range(NCORES)]  # bf16 [RPAD, D]

    def assemble(souts):
        full = np.zeros((N, D), BF16)
        for c in range(NCORES):
            full[row_of[c][valid[c]]] = souts[c][valid[c]]
        return full

    # ---- hop 1: S1 = L @ h ----
    s1_parts = run_hop(h_bf)
    s1 = assemble(s1_parts)
    # ---- hop 2: S2 = L @ S1 ----
    s2_parts = run_hop(s1)

    # ---- dense: outT[t] = sum_k W'[t,k].T @ SkT ----
    def padT(rows_bf):
        out = np.zeros((D, RPAD_D), BF16)
        out[:, :rows_bf.shape[0]] = rows_bf.T
        return out

    in3 = []
    for c in range(NCORES):
        xc = np.where(valid[c][:, None], h[clamped[c]], 0.0).astype(BF16)
        in3.append({"xT": padT(xc), "s1T": padT(s1_parts[c]),
                    "s2T": padT(s2_parts[c]), "wp": wp})
    r3 = run_bass_kernel_spmd(nc_d, in3, core_ids=list(range(NCORES)))

    out = np.empty((T, N, C), np.float32)
    for c in range(NCORES):
        ot = np.asarray(r3.results[c]["outT"], dtype=np.float32)
        out[:, row_of[c][valid[c]], :] = \
            ot[:, :, :RPAD][:, :, valid[c]].transpose(0, 2, 1)
    out += bias[:, None, :]
    return out



# revision 24
# speedup vs baseline: 1.0357x; 1.0357x over previous
"""Trainium2 Bass kernel for nn_BatchedChebLayer (gnn_message_passing).

Strategy (8 NeuronCores, SPMD, 2 launches):
- Flatten features: h = x.transpose(1,0,2).reshape(N, T*C) = [50000, 512],
  cast bf16. Chebyshev: out = x@W0' + S1@W1 + S2@(2W2) + bias with
  S1 = L@h, S2 = L@S1, W0' = W0-W2 (host-folded).
- Core c owns 6250 output rows packed (v2: 2D-degree-balanced greedy,
  minimizes gather-slot padding) into 49 windows of 128. Each SPMM hop:
  edges packed into 128-token blocks per (window, table-half); blocks
  gathered 4-at-a-time (512 idxs/call -- measured optimum; larger calls
  hit a Q7 scratch cliff) with gpsimd.dma_gather (int16 idxs, bf16 rows,
  4 SWDGE queues round-robin). Table split at row 32768 into two DRAM
  tensors so indices fit int16. The hop is Q7-descriptor-generation
  bound: ~994ns/call + ~2.9ns/idx serial on the gpsimd cluster; DMA
  (~450us) hides under generation (~505us) with 4 queues.
- Selection matrices (scatter-free accumulation) built ON DEVICE:
  sel[p,m] = (iota[m] == lrow[p]) * val[p] via one fused DVE
  tensor_scalar(is_equal, mult) per block; psum accumulates f32.
- Launch 1 (build_spmm): S1 rows -> sout (bf16). Host reassembles the
  global S1 table between launches (permutation only).
- Launch 2 (build_spmm_fused): computes S2 in psum AND the whole dense
  stage per window (tensor-engine transposes of x/S1/S2 slices + 12
  [128x128] matmuls into an outT psum tile), writing only outw; the
  separate dense launch is eliminated (its DMA/tensor work hides under
  hop-2's Q7 wall).
"""
import sys
sys.path.insert(0, "/opt/trn_rl_repo")
sys.path.insert(0, "/root/.axon_site/_ro/trn_rl_repo")
import numpy as np
import ml_dtypes

T, N, E, C, KCH = 4, 50000, 800000, 128, 3
D = T * C                  # 512 flat features
NCORES = 8
RPC = N // NCORES          # 6250 rows per core
PTILES = (RPC + 127) // 128            # 49 psum tiles (windows) per core
RPAD = PTILES * 128                    # 6272 padded rows per core
SPLIT = 32768                          # table halves: [0,SPLIT), [SPLIT,N)
NHI = N - SPLIT                        # 17232
QNQ = 4
GB = 4                                 # blocks per dma_gather call
NIPC = GB * 128
CPW = NIPC // 16                       # idx cols per call
SP = True                              # dma_gather single_packet
GBUFS = 16
SELMODE = "dve"                        # "dve" or "dma"
PSBUFS = 6
SELBUFS = 6


def QN(i):
    return i % QNQ


def QMAP(s, i):
    if QSPLIT is None:
        return i % QNQ
    qs = QSPLIT[s]
    return qs[i % len(qs)]


QSPLIT = None                          # e.g. ([0,1,2],[3]) = per-stream queues
DW = 512                               # dense psum width
DTILES = 13
RPAD_D = DTILES * DW                   # 6656

_cache = {}
BF16 = ml_dtypes.bfloat16


PACK = "v2"


def _pack_v2(edge_row, edge_col):
    """Window/core assignment that minimizes gather-slot padding.

    Pads come from ceil(max_core cnt[c,w,s]/128) per (window, stream).
    Greedy 2D packing: (a) deal nodes into 49 window groups of 1024 with
    (lo, hi) degree sums steered toward per-core multiples of 128 minus a
    safety margin; (b) within each group, deal 128 nodes per core with
    LPT balancing of both degree sums.
    """
    dlo = np.bincount(edge_row[edge_col < SPLIT], minlength=N)
    dhi = np.bincount(edge_row[edge_col >= SPLIT], minlength=N)
    nwin = PTILES
    margin = 24
    tgt_lo = np.zeros(nwin, np.int64)
    tgt_hi = np.zeros(nwin, np.int64)
    for tgt, tot in ((tgt_lo, int(dlo.sum())), (tgt_hi, int(dhi.sum()))):
        per_core = tot / NCORES
        ksum = int(np.ceil((per_core + nwin * margin) / 128))
        kbase = ksum // nwin
        kextra = ksum - kbase * nwin
        ks = np.full(nwin, kbase)
        ks[:kextra] += 1
        tgt[:] = NCORES * (ks * 128 - margin)
    # (a) nodes desc by total degree -> window with most remaining 2D slack
    # (maximin of the two normalized slacks keeps lo AND hi on target)
    order = np.argsort(-(dlo + dhi), kind="stable")
    cur_lo = np.zeros(nwin, np.int64)
    cur_hi = np.zeros(nwin, np.int64)
    nnode = np.zeros(nwin, np.int64)
    win_of = np.empty(N, np.int64)
    nlo = max(float(tgt_lo.max()), 1.0)
    nhi = max(float(tgt_hi.max()), 1.0)
    for n in order:
        slack = np.minimum((tgt_lo - cur_lo - dlo[n]) / nlo,
                           (tgt_hi - cur_hi - dhi[n]) / nhi)
        slack[nnode >= NCORES * 128] = -np.inf
        w = int(np.argmax(slack))
        win_of[n] = w
        cur_lo[w] += dlo[n]
        cur_hi[w] += dhi[n]
        nnode[w] += 1
    # (b) within each window, LPT over cores on (lo, hi)
    core_of = np.empty(N, np.int64)
    lrow_of = np.empty(N, np.int64)
    for w in range(nwin):
        nodes = np.where(win_of == w)[0]
        nodes = nodes[np.argsort(-(dlo[nodes] + dhi[nodes]), kind="stable")]
        tlo_pc = max(float(cur_lo[w]) / NCORES, 1.0)
        thi_pc = max(float(cur_hi[w]) / NCORES, 1.0)
        clo = np.zeros(NCORES, np.int64)
        chi = np.zeros(NCORES, np.int64)
        cnt = np.zeros(NCORES, np.int64)
        for n in nodes:
            load = np.maximum((clo + dlo[n]) / tlo_pc,
                              (chi + dhi[n]) / thi_pc)
            load[cnt >= 128] = np.inf
            c = int(np.argmin(load))
            core_of[n] = c
            lrow_of[n] = w * 128 + cnt[c]
            clo[c] += dlo[n]
            chi[c] += dhi[n]
            cnt[c] += 1
    return core_of, lrow_of


def build_schedule(edge_row, edge_col, edge_val, gb=None):
    """Degree-balanced deal + per-(window, table-half) block packing.

    Returns dict with:
      row_of   [NCORES, RPAD] global row id per (core, local row), -1 pad
      blocks   list per tile w: (BLO, BHI)
      tilemeta list per tile w: list of (stream, call, kk) per block
      ncalls   (ncalls_lo, ncalls_hi)
      idx_lo/hi [NCORES, 128, ncalls*64] int16
      lrow     [NCORES, 128, NB] f32   (target row within window, 0..127)
      val      [NCORES, 128, NB] f32   (edge weight; 0 for pad slots)
      NB       total blocks per tile-sweep (one hop)
    """
    gb = GB if gb is None else gb
    nipc = gb * 128
    cpw = nipc // 16
    order = np.argsort(edge_row, kind="stable")
    rows = edge_row[order].astype(np.int64)
    cols = edge_col[order].astype(np.int64)
    vals = edge_val[order].astype(np.float32)

    nwin = PTILES
    if PACK == "v2":
        core_of, lrow_of = _pack_v2(edge_row, edge_col)
    else:
        deg = np.bincount(edge_row, minlength=N)
        srows = np.argsort(-deg, kind="stable")
        nbuck = NCORES * nwin
        pos = np.arange(N)
        p, j = pos // nbuck, pos % nbuck
        buck = np.where(p % 2 == 0, j, nbuck - 1 - j)
        core_of = np.empty(N, np.int64)
        lrow_of = np.empty(N, np.int64)
        core_of[srows] = buck % NCORES
        win_of = buck // NCORES
        occ = np.zeros(N, np.int64)
        sort_b = np.argsort(buck, kind="stable")
        bs = buck[sort_b]
        starts = np.searchsorted(bs, np.arange(nbuck), side="left")
        occ[sort_b] = np.arange(N) - starts[bs]
        assert occ.max() < 128, "bucket overflow"
        lrow_of[srows] = win_of * 128 + occ
    row_of = np.full((NCORES, RPAD), -1, np.int64)
    row_of[core_of, lrow_of] = np.arange(N)

    # per-token: core, window, local row m, stream, local col idx
    tcore = core_of[rows]
    tlrow = lrow_of[rows]
    twin = tlrow // 128
    tm = tlrow % 128
    tstream = (cols >= SPLIT).astype(np.int64)
    tjloc = np.where(tstream == 0, cols, cols - SPLIT)

    # counts per (core, window, stream)
    key = (tcore * nwin + twin) * 2 + tstream
    cnt = np.bincount(key, minlength=NCORES * nwin * 2).reshape(
        NCORES, nwin, 2)
    bw = -(-cnt.max(axis=0) // 128)          # [nwin, 2] blocks per stream
    BLO, BHI = bw[:, 0], bw[:, 1]
    NB = int(bw.sum())

    # tile-local block ordinals and global stream block indices
    lo_base = np.zeros(nwin + 1, np.int64)
    np.cumsum(BLO, out=lo_base[1:])
    hi_base = np.zeros(nwin + 1, np.int64)
    np.cumsum(BHI, out=hi_base[1:])
    tile_base = np.zeros(nwin + 1, np.int64)
    np.cumsum(BLO + BHI, out=tile_base[1:])
    ncalls_lo = int(-(-lo_base[-1] // gb))
    ncalls_hi = int(-(-hi_base[-1] // gb))

    tilemeta = []
    for w in range(nwin):
        meta = []
        for k in range(int(BLO[w])):
            sbi = lo_base[w] + k
            meta.append((0, int(sbi // gb), int(sbi % gb)))
        for k in range(int(BHI[w])):
            sbi = hi_base[w] + k
            meta.append((1, int(sbi // gb), int(sbi % gb)))
        tilemeta.append(meta)

    # Prefill pad slots with spread row ids: all-zero pad idxs all hit HBM
    # row 0, serializing on one bank (measured 5x slower when degenerate).
    def _spread(ncalls, lim):
        a = (np.arange(ncalls * cpw * 16, dtype=np.int64) * 97) % lim
        return np.broadcast_to(
            a.astype(np.int16).reshape(1, ncalls * cpw, 16).transpose(0, 2, 1),
            (NCORES, 16, ncalls * cpw)).copy()

    idx_lo = _spread(ncalls_lo, SPLIT)
    idx_hi = _spread(ncalls_hi, NHI)
    lrow = np.zeros((NCORES, 128, NB), np.float32)
    val = np.zeros((NCORES, 128, NB), np.float32)

    # slot assignment: stable-sort tokens by (core, window, stream)
    sort2 = np.argsort(key, kind="stable")
    ks = key[sort2]
    gstarts = np.searchsorted(ks, np.arange(NCORES * nwin * 2), side="left")
    q = np.arange(E) - gstarts[ks]           # slot within (c,w,s) group
    c2 = tcore[sort2]
    w2 = twin[sort2]
    s2 = tstream[sort2]
    m2 = tm[sort2]
    j2 = tjloc[sort2]
    v2 = vals[sort2]
    kblk = q // 128                          # block within (w, s)
    pslot = q % 128
    # tile-local block ordinal
    bo = np.where(s2 == 0, kblk, BLO[w2] + kblk) + tile_base[w2]
    lrow[c2, pslot, bo] = m2
    val[c2, pslot, bo] = v2
    # gather stream position
    sbi = np.where(s2 == 0, lo_base[w2], hi_base[w2]) + kblk
    cidx = sbi // gb
    kk = sbi % gb
    gi = kk * 128 + pslot                    # index within call
    gcol = cidx * cpw + gi // 16
    gpart = gi % 16
    idx16 = j2.astype(np.int16)
    m_lo = s2 == 0
    idx_lo[c2[m_lo], gpart[m_lo], gcol[m_lo]] = idx16[m_lo]
    m_hi = ~m_lo
    idx_hi[c2[m_hi], gpart[m_hi], gcol[m_hi]] = idx16[m_hi]
    # replicate the 16-partition index groups x8 (one copy per Q7 core)
    idx_lo = np.ascontiguousarray(np.tile(idx_lo, (1, 8, 1)))
    idx_hi = np.ascontiguousarray(np.tile(idx_hi, (1, 8, 1)))

    assert max(len(m) for m in tilemeta) <= 24, "sel tile too small"
    assert min(len(m) for m in tilemeta) >= 1
    blocks = [(int(BLO[w]), int(BHI[w])) for w in range(nwin)]
    return dict(row_of=row_of, blocks=blocks, tilemeta=tilemeta,
                ncalls=(ncalls_lo, ncalls_hi), idx_lo=idx_lo, idx_hi=idx_hi,
                lrow=lrow, val=val, NB=NB, gb=gb, cpw=cpw)


def build_spmm(sched, repeat=1, gbufs=None, selbufs=None, psbufs=None,
               skip_mm=False, skip_gather=False, skip_sel=False,
               nqueues=4, scratch=None):
    import concourse.bacc as bacc
    import concourse.tile as tile
    import concourse.mybir as mybir
    from concourse import library_config

    gb = sched["gb"]
    cpw = sched["cpw"]
    gbufs = GBUFS if gbufs is None else gbufs
    selbufs = SELBUFS if selbufs is None else selbufs
    psbufs = PSBUFS if psbufs is None else psbufs
    ncalls_lo, ncalls_hi = sched["ncalls"]
    NBt = sched["NB"]
    tilemeta = sched["tilemeta"]
    selmax = max(len(m) for m in tilemeta)
    kw = {} if scratch is None else dict(dynamic_dma_scratch_size=scratch)
    nc = bacc.Bacc("TRN2", target_bir_lowering=False, debug=False,
                   num_devices=NCORES, num_swdge_queues=nqueues, **kw)
    bf = mybir.dt.bfloat16
    tlo = nc.dram_tensor("tlo", [SPLIT, D], bf, kind="ExternalInput")
    thi = nc.dram_tensor("thi", [NHI, D], bf, kind="ExternalInput")
    idxlo = nc.dram_tensor("idxlo", [128, ncalls_lo * cpw], mybir.dt.int16,
                           kind="ExternalInput")
    idxhi = nc.dram_tensor("idxhi", [128, ncalls_hi * cpw], mybir.dt.int16,
                           kind="ExternalInput")
    lrow = nc.dram_tensor("lrow", [128, NBt], mybir.dt.float32,
                          kind="ExternalInput")
    val = nc.dram_tensor("val", [128, NBt], mybir.dt.float32,
                         kind="ExternalInput")
    iota = nc.dram_tensor("iota", [128, 128], mybir.dt.float32,
                          kind="ExternalInput")
    if SELMODE == "dma":
        seldram = nc.dram_tensor("sel", [128, NBt * 128], bf,
                                 kind="ExternalInput")
    sout = nc.dram_tensor("sout", [RPAD, D], bf, kind="ExternalOutput")
    tabs = [tlo, thi]
    idxts = []
    with tile.TileContext(nc) as tc:
        with (
            tc.tile_pool(name="const", bufs=1) as cpool,
            tc.tile_pool(name="glo", bufs=gbufs) as glopool,
            tc.tile_pool(name="ghi", bufs=gbufs) as ghipool,
            tc.tile_pool(name="selp", bufs=selbufs) as selpool,
            tc.tile_pool(name="stp", bufs=3) as stpool,
            tc.tile_pool(name="psum", bufs=psbufs, space="PSUM") as ppool,
        ):
            nc.gpsimd.load_library(library_config.mlp)
            it_lo = cpool.tile([128, ncalls_lo * cpw], mybir.dt.int16)
            nc.sync.dma_start(it_lo[:], idxlo[:])
            it_hi = cpool.tile([128, ncalls_hi * cpw], mybir.dt.int16)
            nc.sync.dma_start(it_hi[:], idxhi[:])
            lrow_t = cpool.tile([128, NBt], mybir.dt.float32)
            nc.sync.dma_start(lrow_t[:], lrow[:])
            val_t = cpool.tile([128, NBt], mybir.dt.float32)
            nc.sync.dma_start(val_t[:], val[:])
            iota_t = cpool.tile([128, 128], mybir.dt.float32)
            nc.sync.dma_start(iota_t[:], iota[:])
            idxts = [it_lo, it_hi]
            gpools = [glopool, ghipool]
            total_calls = [ncalls_lo, ncalls_hi]
            nblocks_stream = [0, 0]
            for meta in tilemeta:
                for (s, cidx, kk) in meta:
                    nblocks_stream[s] = max(nblocks_stream[s], cidx * gb + kk + 1)
            with tc.For_i(0, repeat):
                gtiles = [{}, {}]
                qctr = [0]

                def issue(s, cidx):
                    nb_in_call = min(gb, nblocks_stream[s] - cidx * gb)
                    g = gpools[s].tile([128, gb * D], mybir.dt.bfloat16,
                                       tag=f"g{s}")
                    if not skip_gather:
                        ni = nb_in_call * 128
                        gv = g[:, :nb_in_call * D].rearrange(
                            "p (k e) -> p k e", k=nb_in_call)
                        qn = (qctr[0] % nqueues if QSPLIT is None
                              else QMAP(s, qctr[0]))
                        nc.gpsimd.dma_gather(
                            gv, tabs[s][:],
                            idxts[s][:, cidx * cpw:cidx * cpw + nb_in_call * 8],
                            ni, ni, D, queue_num=qn,
                            single_packet=SP)
                    qctr[0] += 1
                    gtiles[s][cidx] = g

                bo = 0
                for w in range(PTILES):
                    meta = tilemeta[w]
                    nbw = len(meta)
                    sel_t = selpool.tile([128, selmax * 128],
                                         mybir.dt.bfloat16, tag="sel")
                    ps = ppool.tile([128, D], mybir.dt.float32)
                    if SELMODE == "dma":
                        nc.sync.dma_start(
                            sel_t[:, :nbw * 128],
                            seldram[:, bo * 128:(bo + nbw) * 128])
                    for bi, (s, cidx, kk) in enumerate(meta):
                        if cidx not in gtiles[s]:
                            issue(s, cidx)
                        if SELMODE == "dve" and not skip_sel:
                            nc.vector.tensor_scalar(
                                sel_t[:, bi * 128:(bi + 1) * 128],
                                iota_t[:],
                                lrow_t[:, bo + bi:bo + bi + 1],
                                val_t[:, bo + bi:bo + bi + 1],
                                mybir.AluOpType.is_equal,
                                mybir.AluOpType.mult)
                        if not skip_mm:
                            nc.tensor.matmul(
                                out=ps[:],
                                lhsT=sel_t[:, bi * 128:(bi + 1) * 128],
                                rhs=gtiles[s][cidx][:, kk * D:(kk + 1) * D],
                                start=(bi == 0), stop=(bi == nbw - 1))
                    st = stpool.tile([128, D], mybir.dt.bfloat16, tag="st")
                    if skip_mm:
                        nc.scalar.copy(st[:], sel_t[:, :D])
                    else:
                        nc.scalar.copy(st[:], ps[:])
                    nc.sync.dma_start(sout[w * 128:(w + 1) * 128, :], st[:])
                    bo += nbw
    nc.compile()
    return nc


def build_spmm_fused(sched, repeat=1, gbufs=None, selbufs=None, psbufs=None):
    """Hop-2 spmm with the dense stage fused in.

    Per window w: psum S2_w as in build_spmm, then
      outT_w[fo, t*128+r] = sum_k W'[t,k]^T @ {x, S1, S2}T_w[t]
    with the three rhs operands produced by on-device tensor-engine
    transposes of [128,128] slices (x/S1 window rows loaded contiguously
    from xloc/s1loc; S2 from the st bf16 copy). Drops the sout write;
    writes outw [128, PTILES*D] instead. x W0' uses W0-W2, S2 term 2*W2
    (host-folded wp, same as build_dense).
    """
    import concourse.bacc as bacc
    import concourse.tile as tile
    import concourse.mybir as mybir
    from concourse import library_config

    gb = sched["gb"]
    cpw = sched["cpw"]
    gbufs = 14 if gbufs is None else gbufs
    selbufs = SELBUFS if selbufs is None else selbufs
    psbufs = 4 if psbufs is None else psbufs
    ncalls_lo, ncalls_hi = sched["ncalls"]
    NBt = sched["NB"]
    tilemeta = sched["tilemeta"]
    selmax = max(len(m) for m in tilemeta)
    nc = bacc.Bacc("TRN2", target_bir_lowering=False, debug=False,
                   num_devices=NCORES, num_swdge_queues=4)
    bf = mybir.dt.bfloat16
    f32 = mybir.dt.float32
    tlo = nc.dram_tensor("tlo", [SPLIT, D], bf, kind="ExternalInput")
    thi = nc.dram_tensor("thi", [NHI, D], bf, kind="ExternalInput")
    idxlo = nc.dram_tensor("idxlo", [128, ncalls_lo * cpw], mybir.dt.int16,
                           kind="ExternalInput")
    idxhi = nc.dram_tensor("idxhi", [128, ncalls_hi * cpw], mybir.dt.int16,
                           kind="ExternalInput")
    lrow = nc.dram_tensor("lrow", [128, NBt], f32, kind="ExternalInput")
    val = nc.dram_tensor("val", [128, NBt], f32, kind="ExternalInput")
    iota = nc.dram_tensor("iota", [128, 128], f32, kind="ExternalInput")
    xloc = nc.dram_tensor("xloc", [RPAD, D], bf, kind="ExternalInput")
    s1loc = nc.dram_tensor("s1loc", [RPAD, D], bf, kind="ExternalInput")
    wp = nc.dram_tensor("wp", [C, T * KCH * C], bf, kind="ExternalInput")
    ident = nc.dram_tensor("ident", [128, 128], bf, kind="ExternalInput")
    outw = nc.dram_tensor("outw", [128, PTILES * D], bf,
                          kind="ExternalOutput")
    tabs = [tlo, thi]
    with tile.TileContext(nc) as tc:
        with (
            tc.tile_pool(name="const", bufs=1) as cpool,
            tc.tile_pool(name="glo", bufs=gbufs) as glopool,
            tc.tile_pool(name="ghi", bufs=gbufs) as ghipool,
            tc.tile_pool(name="selp", bufs=selbufs) as selpool,
            tc.tile_pool(name="stp", bufs=2) as stpool,
            tc.tile_pool(name="locp", bufs=2) as locpool,
            tc.tile_pool(name="xtp", bufs=2) as xtpool,
            tc.tile_pool(name="otp", bufs=2) as otpool,
            tc.tile_pool(name="psum", bufs=psbufs, space="PSUM") as ppool,
            tc.tile_pool(name="pstr", bufs=2, space="PSUM") as trpool,
            tc.tile_pool(name="psot", bufs=2, space="PSUM") as otppool,
        ):
            nc.gpsimd.load_library(library_config.mlp)
            it_lo = cpool.tile([128, ncalls_lo * cpw], mybir.dt.int16)
            nc.sync.dma_start(it_lo[:], idxlo[:])
            it_hi = cpool.tile([128, ncalls_hi * cpw], mybir.dt.int16)
            nc.sync.dma_start(it_hi[:], idxhi[:])
            lrow_t = cpool.tile([128, NBt], f32)
            nc.sync.dma_start(lrow_t[:], lrow[:])
            val_t = cpool.tile([128, NBt], f32)
            nc.sync.dma_start(val_t[:], val[:])
            iota_t = cpool.tile([128, 128], f32)
            nc.sync.dma_start(iota_t[:], iota[:])
            w_t = cpool.tile([128, T * KCH * C], bf)
            nc.sync.dma_start(w_t[:], wp[:])
            id_t = cpool.tile([128, 128], bf)
            nc.sync.dma_start(id_t[:], ident[:])
            idxts = [it_lo, it_hi]
            gpools = [glopool, ghipool]
            nblocks_stream = [0, 0]
            for meta in tilemeta:
                for (s, cidx, kk) in meta:
                    nblocks_stream[s] = max(nblocks_stream[s],
                                            cidx * gb + kk + 1)
            with tc.For_i(0, repeat):
                gtiles = [{}, {}]
                qctr = [0]

                def issue(s, cidx):
                    nb_in_call = min(gb, nblocks_stream[s] - cidx * gb)
                    g = gpools[s].tile([128, gb * D], bf, tag=f"g{s}")
                    ni = nb_in_call * 128
                    gv = g[:, :nb_in_call * D].rearrange(
                        "p (k e) -> p k e", k=nb_in_call)
                    nc.gpsimd.dma_gather(
                        gv, tabs[s][:],
                        idxts[s][:, cidx * cpw:cidx * cpw + nb_in_call * 8],
                        ni, ni, D, queue_num=QMAP(s, qctr[0]),
                        single_packet=SP)
                    qctr[0] += 1
                    gtiles[s][cidx] = g

                bo = 0
                for w in range(PTILES):
                    meta = tilemeta[w]
                    nbw = len(meta)
                    sel_t = selpool.tile([128, selmax * 128], bf, tag="sel")
                    ps = ppool.tile([128, D], f32)
                    xw = locpool.tile([128, D], bf, tag="xw")
                    nc.sync.dma_start(xw[:], xloc[w * 128:(w + 1) * 128, :])
                    s1w = locpool.tile([128, D], bf, tag="s1w")
                    nc.sync.dma_start(s1w[:], s1loc[w * 128:(w + 1) * 128, :])
                    for bi, (s, cidx, kk) in enumerate(meta):
                        if cidx not in gtiles[s]:
                            issue(s, cidx)
                        nc.vector.tensor_scalar(
                            sel_t[:, bi * 128:(bi + 1) * 128],
                            iota_t[:],
                            lrow_t[:, bo + bi:bo + bi + 1],
                            val_t[:, bo + bi:bo + bi + 1],
                            mybir.AluOpType.is_equal,
                            mybir.AluOpType.mult)
                        nc.tensor.matmul(
                            out=ps[:],
                            lhsT=sel_t[:, bi * 128:(bi + 1) * 128],
                            rhs=gtiles[s][cidx][:, kk * D:(kk + 1) * D],
                            start=(bi == 0), stop=(bi == nbw - 1))
                    st = stpool.tile([128, D], bf, tag="st")
                    nc.scalar.copy(st[:], ps[:])
                    # dense fusion: build XT slices via tensor transposes
                    xt_all = xtpool.tile([128, 3 * D], bf, tag="xt")
                    for si, src in enumerate((xw, s1w, st)):
                        tp = trpool.tile([128, D], bf, tag="tp")
                        for t in range(T):
                            nc.tensor.transpose(
                                tp[:, t * C:(t + 1) * C],
                                src[:, t * C:(t + 1) * C], id_t[:])
                        nc.vector.tensor_copy(
                            xt_all[:, si * D:(si + 1) * D], tp[:])
                    ot_ps = otppool.tile([128, D], f32)
                    for t in range(T):
                        for k in range(KCH):
                            nc.tensor.matmul(
                                out=ot_ps[:, t * C:(t + 1) * C],
                                lhsT=w_t[:, (t * KCH + k) * C:
                                         (t * KCH + k + 1) * C],
                                rhs=xt_all[:, k * D + t * C:
                                           k * D + (t + 1) * C],
                                start=(k == 0), stop=(k == KCH - 1))
                    ot = otpool.tile([128, D], bf, tag="ot")
                    nc.scalar.copy(ot[:], ot_ps[:])
                    nc.sync.dma_start(outw[:, w * D:(w + 1) * D], ot[:])
                    bo += nbw
    nc.compile()
    return nc


def build_dense(repeat=1):
    import concourse.bacc as bacc
    import concourse.tile as tile
    import concourse.mybir as mybir

    nc = bacc.Bacc("TRN2", target_bir_lowering=False, debug=False,
                   num_devices=NCORES)
    bf = mybir.dt.bfloat16
    xT = nc.dram_tensor("xT", [D, RPAD_D], bf, kind="ExternalInput")
    s1T = nc.dram_tensor("s1T", [D, RPAD_D], bf, kind="ExternalInput")
    s2T = nc.dram_tensor("s2T", [D, RPAD_D], bf, kind="ExternalInput")
    wp = nc.dram_tensor("wp", [C, T * KCH * C], bf, kind="ExternalInput")
    outT = nc.dram_tensor("outT", [T, C, RPAD_D], bf, kind="ExternalOutput")
    srcs = [xT, s1T, s2T]
    with tile.TileContext(nc) as tc:
        with (
            tc.tile_pool(name="wpool", bufs=1) as wpool,
            tc.tile_pool(name="rhsp", bufs=2) as rhspool,
            tc.tile_pool(name="outp", bufs=2) as outpool,
            tc.tile_pool(name="psum", bufs=4, space="PSUM") as ppool,
        ):
            w_t = wpool.tile([128, T * KCH * C], bf)
            nc.sync.dma_start(w_t[:], wp[:])
            with tc.For_i(0, repeat):
                for t in range(T):
                    rhss = []
                    for k in range(KCH):
                        rhs = rhspool.tile([128, RPAD_D], bf, tag=f"rhs{k}")
                        nc.sync.dma_start(rhs[:], srcs[k][t * C:(t + 1) * C, :])
                        rhss.append(rhs)
                    ot = outpool.tile([128, RPAD_D], bf, tag="ot")
                    for dw in range(DTILES):
                        ps = ppool.tile([128, DW], mybir.dt.float32)
                        for k in range(KCH):
                            nc.tensor.matmul(
                                out=ps[:],
                                lhsT=w_t[:, (t * KCH + k) * C:(t * KCH + k + 1) * C],
                                rhs=rhss[k][:, dw * DW:(dw + 1) * DW],
                                start=(k == 0), stop=(k == KCH - 1))
                        nc.vector.tensor_copy(ot[:, dw * DW:(dw + 1) * DW], ps[:])
                    nc.sync.dma_start(outT[t, :, :], ot[:])
    nc.compile()
    return nc


_IOTA = np.tile(np.arange(128, dtype=np.float32), (128, 1))


def host_sel(sched):
    if "hostsel" in sched:
        return sched["hostsel"]
    NB = sched["NB"]
    sel = np.zeros((NCORES, 128, NB, 128), BF16)
    li = sched["lrow"].astype(np.int64)
    cc, pp, bb = np.meshgrid(np.arange(NCORES), np.arange(128),
                             np.arange(NB), indexing="ij")
    sel[cc, pp, bb, li] = sched["val"].astype(BF16)
    sched["hostsel"] = sel.reshape(NCORES, 128, NB * 128)
    return sched["hostsel"]


def _spmm_inputs(sched, tab_bf):
    """Per-core input dicts for one hop given the bf16 table [N, D]."""
    tlo = np.ascontiguousarray(tab_bf[:SPLIT])
    thi = np.ascontiguousarray(tab_bf[SPLIT:])
    ins = [
        {"tlo": tlo, "thi": thi,
         "idxlo": sched["idx_lo"][c], "idxhi": sched["idx_hi"][c],
         "lrow": sched["lrow"][c], "val": sched["val"][c], "iota": _IOTA}
        for c in range(NCORES)]
    if SELMODE == "dma":
        hs = host_sel(sched)
        for c in range(NCORES):
            ins[c]["sel"] = hs[c]
    return ins


_IDENT = np.eye(128, dtype=BF16)


def _spmm2_inputs(sched, tab_bf, xlocs, s1locs, wp):
    """Per-core input dicts for the fused hop-2 program."""
    ins = _spmm_inputs(sched, tab_bf)
    for c in range(NCORES):
        ins[c].update({"xloc": xlocs[c], "s1loc": np.asarray(s1locs[c]),
                       "wp": wp, "ident": _IDENT})
    return ins


def kernel(x, edge_row, edge_col, edge_val, weight, bias):
    from concourse.bass_utils import run_bass_kernel_spmd

    x = np.asarray(x, dtype=np.float32)
    edge_row = np.asarray(edge_row).astype(np.int64)
    edge_col = np.asarray(edge_col).astype(np.int64)
    edge_val = np.asarray(edge_val, dtype=np.float32)
    weight = np.asarray(weight, dtype=np.float32)
    bias = np.asarray(bias, dtype=np.float32)

    h = np.ascontiguousarray(x.transpose(1, 0, 2).reshape(N, D))  # [N, T*C]
    h_bf = h.astype(BF16)
    wp = np.stack(
        [weight[:, 0] - weight[:, 2], weight[:, 1], 2.0 * weight[:, 2]],
        axis=1)  # [T, 3, C, C]
    wp = np.ascontiguousarray(
        wp.transpose(2, 0, 1, 3).reshape(C, T * KCH * C)).astype(BF16)

    fp = (E, int(edge_row[:64].sum()), int(edge_col[:64].sum()),
          int(edge_row[-64:].sum()), int(edge_col[-64:].sum()))
    if _cache.get("fp") != fp:
        _cache.clear()
        _cache["fp"] = fp
        _cache["sched"] = build_schedule(edge_row, edge_col, edge_val)
    sched = _cache["sched"]
    if "spmm" not in _cache:
        _cache["spmm"] = build_spmm(sched)
    if "spmm2" not in _cache:
        _cache["spmm2"] = build_spmm_fused(sched)
    nc_s, nc_f = _cache["spmm"], _cache["spmm2"]
    row_of = sched["row_of"]
    valid = row_of >= 0
    clamped = np.maximum(row_of, 0)

    # ---- hop 1: S1 = L @ h ----
    r1 = run_bass_kernel_spmd(nc_s, _spmm_inputs(sched, h_bf),
                              core_ids=list(range(NCORES)))
    s1_parts = [r1.results[c]["sout"] for c in range(NCORES)]  # bf16 [RPAD,D]
    s1 = np.zeros((N, D), BF16)
    for c in range(NCORES):
        s1[row_of[c][valid[c]]] = s1_parts[c][valid[c]]

    # ---- hop 2 (fused): S2 = L @ S1; out = x W0' + S1 W1 + S2 2W2 ----
    xlocs = [np.where(valid[c][:, None], h[clamped[c]], 0.0).astype(BF16)
             for c in range(NCORES)]
    r2 = run_bass_kernel_spmd(
        nc_f, _spmm2_inputs(sched, s1, xlocs, s1_parts, wp),
        core_ids=list(range(NCORES)))

    out = np.empty((T, N, C), np.float32)
    for c in range(NCORES):
        ow = np.asarray(r2.results[c]["outw"], dtype=np.float32)
        loc = ow.reshape(C, PTILES, T, 128).transpose(2, 1, 3, 0).reshape(
            T, RPAD, C)
        out[:, row_of[c][valid[c]], :] = loc[:, valid[c], :]
    out += bias[:, None, :]
    return out



# revision 26
# speedup vs baseline: 1.1940x; 1.1529x over previous
"""Trainium2 Bass kernel for nn_BatchedChebLayer (gnn_message_passing).

Strategy (8 NeuronCores, SPMD, 2 launches):
- Flatten features: h = x.transpose(1,0,2).reshape(N, T*C) = [50000, 512],
  cast bf16. Chebyshev: out = x@W0' + S1@W1 + S2@(2W2) + bias with
  S1 = L@h, S2 = L@S1, W0' = W0-W2 (host-folded).
- Core c owns 6250 output rows packed (v2: 2D-degree-balanced greedy,
  minimizes gather-slot padding) into 49 windows of 128. Each SPMM hop:
  edges packed into 128-token blocks per (window, table-half); blocks
  gathered 4-at-a-time (512 idxs/call -- measured optimum; larger calls
  hit a Q7 scratch cliff) with gpsimd.dma_gather (int16 idxs, bf16 rows,
  4 SWDGE queues round-robin). Table split at row 32768 into two DRAM
  tensors so indices fit int16. The hop is Q7-descriptor-generation
  bound: ~994ns/call + ~2.9ns/idx serial on the gpsimd cluster; DMA
  (~450us) hides under generation (~505us) with 4 queues.
- Selection matrices (scatter-free accumulation) built ON DEVICE:
  sel[p,m] = (iota[m] == lrow[p]) * val[p] via one fused DVE
  tensor_scalar(is_equal, mult) per block; psum accumulates f32.
- Launch 1 (build_spmm): S1 rows -> sout (bf16). Host reassembles the
  global S1 table between launches (permutation only).
- Launch 2 (build_spmm_fused): computes S2 in psum AND the whole dense
  stage per window (tensor-engine transposes of x/S1/S2 slices + 12
  [128x128] matmuls into an outT psum tile), writing only outw; the
  separate dense launch is eliminated (its DMA/tensor work hides under
  hop-2's Q7 wall).
"""
import sys
sys.path.insert(0, "/opt/trn_rl_repo")
sys.path.insert(0, "/root/.axon_site/_ro/trn_rl_repo")
import numpy as np
import ml_dtypes

T, N, E, C, KCH = 4, 50000, 800000, 128, 3
D = T * C                  # 512 flat features
NCORES = 8
RPC = N // NCORES          # 6250 rows per core
PTILES = (RPC + 127) // 128            # 49 psum tiles (windows) per core
RPAD = PTILES * 128                    # 6272 padded rows per core
SPLIT = 32768                          # table halves: [0,SPLIT), [SPLIT,N)
NHI = N - SPLIT                        # 17232
QNQ = 4
GB = 4                                 # blocks per dma_gather call
NIPC = GB * 128
CPW = NIPC // 16                       # idx cols per call
SP = True                              # dma_gather single_packet
GBUFS = 16
SELMODE = "dve"                        # "dve" or "dma"
PSBUFS = 6
SELBUFS = 6


def QN(i):
    return i % QNQ


def QMAP(s, i):
    if QSPLIT is None:
        return i % QNQ
    qs = QSPLIT[s]
    return qs[i % len(qs)]


QSPLIT = None                          # e.g. ([0,1,2],[3]) = per-stream queues
DW = 512                               # dense psum width
DTILES = 13
RPAD_D = DTILES * DW                   # 6656

_cache = {}
BF16 = ml_dtypes.bfloat16


PACK = "v2"


def _pack_v2(edge_row, edge_col):
    """Window/core assignment that minimizes gather-slot padding.

    Pads come from ceil(max_core cnt[c,w,s]/128) per (window, stream).
    Greedy 2D packing: (a) deal nodes into 49 window groups of 1024 with
    (lo, hi) degree sums steered toward per-core multiples of 128 minus a
    safety margin; (b) within each group, deal 128 nodes per core with
    LPT balancing of both degree sums.
    """
    dlo = np.bincount(edge_row[edge_col < SPLIT], minlength=N)
    dhi = np.bincount(edge_row[edge_col >= SPLIT], minlength=N)
    nwin = PTILES
    margin = 24
    tgt_lo = np.zeros(nwin, np.int64)
    tgt_hi = np.zeros(nwin, np.int64)
    for tgt, tot in ((tgt_lo, int(dlo.sum())), (tgt_hi, int(dhi.sum()))):
        per_core = tot / NCORES
        ksum = int(np.ceil((per_core + nwin * margin) / 128))
        kbase = ksum // nwin
        kextra = ksum - kbase * nwin
        ks = np.full(nwin, kbase)
        ks[:kextra] += 1
        tgt[:] = NCORES * (ks * 128 - margin)
    # (a) nodes desc by total degree -> window with most remaining 2D slack
    # (maximin of the two normalized slacks keeps lo AND hi on target)
    order = np.argsort(-(dlo + dhi), kind="stable")
    cur_lo = np.zeros(nwin, np.int64)
    cur_hi = np.zeros(nwin, np.int64)
    nnode = np.zeros(nwin, np.int64)
    win_of = np.empty(N, np.int64)
    nlo = max(float(tgt_lo.max()), 1.0)
    nhi = max(float(tgt_hi.max()), 1.0)
    for n in order:
        slack = np.minimum((tgt_lo - cur_lo - dlo[n]) / nlo,
                           (tgt_hi - cur_hi - dhi[n]) / nhi)
        slack[nnode >= NCORES * 128] = -np.inf
        w = int(np.argmax(slack))
        win_of[n] = w
        cur_lo[w] += dlo[n]
        cur_hi[w] += dhi[n]
        nnode[w] += 1
    # (a2) repair pass: windows whose realized sum sits just over a
    # per-core block boundary get nodes moved to windows with slack
    # (per-core counts track window sums closely, so shaving the sum
    # under boundary*NCORES removes a block for every core).
    for stream, d in ((0, dlo), (1, dhi)):
        for _ in range(3):
            sums = np.zeros(nwin, np.int64)
            np.add.at(sums, win_of, d)
            mod = (sums // NCORES) % 128
            moved = False
            for w in np.where((mod > 0) & (mod <= 20))[0]:
                excess = int(sums[w] - (sums[w] // (NCORES * 128))
                             * NCORES * 128)
                cand = np.where((win_of == w) & (d > 0)
                                & (d <= max(excess // 4, 4)))[0]
                room = ((sums // NCORES) % 128 < 96) & (nnode < NCORES * 128)
                tw = np.where(room)[0]
                if not len(tw):
                    continue
                take = 0
                for n in cand:
                    if take >= excess:
                        break
                    dst = int(tw[np.argmin(sums[tw])])
                    if nnode[dst] >= NCORES * 128:
                        tw = tw[tw != dst]
                        if not len(tw):
                            break
                        continue
                    nnode[w] -= 1
                    nnode[dst] += 1
                    sums[w] -= d[n]
                    sums[dst] += d[n]
                    win_of[n] = dst
                    take += d[n]
                    moved = True
            if not moved:
                break
    # (b) within each window, LPT over cores on (lo, hi)
    core_of = np.empty(N, np.int64)
    lrow_of = np.empty(N, np.int64)
    for w in range(nwin):
        nodes = np.where(win_of == w)[0]
        nodes = nodes[np.argsort(-(dlo[nodes] + dhi[nodes]), kind="stable")]
        tlo_pc = max(float(cur_lo[w]) / NCORES, 1.0)
        thi_pc = max(float(cur_hi[w]) / NCORES, 1.0)
        clo = np.zeros(NCORES, np.int64)
        chi = np.zeros(NCORES, np.int64)
        cnt = np.zeros(NCORES, np.int64)
        for n in nodes:
            load = np.maximum((clo + dlo[n]) / tlo_pc,
                              (chi + dhi[n]) / thi_pc)
            load[cnt >= 128] = np.inf
            c = int(np.argmin(load))
            core_of[n] = c
            lrow_of[n] = w * 128 + cnt[c]
            clo[c] += dlo[n]
            chi[c] += dhi[n]
            cnt[c] += 1
    return core_of, lrow_of


def build_schedule(edge_row, edge_col, edge_val, gb=None):
    """Degree-balanced deal + per-(window, table-half) block packing.

    Returns dict with:
      row_of   [NCORES, RPAD] global row id per (core, local row), -1 pad
      blocks   list per tile w: (BLO, BHI)
      tilemeta list per tile w: list of (stream, call, kk) per block
      ncalls   (ncalls_lo, ncalls_hi)
      idx_lo/hi [NCORES, 128, ncalls*64] int16
      lrow     [NCORES, 128, NB] f32   (target row within window, 0..127)
      val      [NCORES, 128, NB] f32   (edge weight; 0 for pad slots)
      NB       total blocks per tile-sweep (one hop)
    """
    gb = GB if gb is None else gb
    nipc = gb * 128
    cpw = nipc // 16
    order = np.argsort(edge_row, kind="stable")
    rows = edge_row[order].astype(np.int64)
    cols = edge_col[order].astype(np.int64)
    vals = edge_val[order].astype(np.float32)

    nwin = PTILES
    if PACK == "v2":
        core_of, lrow_of = _pack_v2(edge_row, edge_col)
    else:
        deg = np.bincount(edge_row, minlength=N)
        srows = np.argsort(-deg, kind="stable")
        nbuck = NCORES * nwin
        pos = np.arange(N)
        p, j = pos // nbuck, pos % nbuck
        buck = np.where(p % 2 == 0, j, nbuck - 1 - j)
        core_of = np.empty(N, np.int64)
        lrow_of = np.empty(N, np.int64)
        core_of[srows] = buck % NCORES
        win_of = buck // NCORES
        occ = np.zeros(N, np.int64)
        sort_b = np.argsort(buck, kind="stable")
        bs = buck[sort_b]
        starts = np.searchsorted(bs, np.arange(nbuck), side="left")
        occ[sort_b] = np.arange(N) - starts[bs]
        assert occ.max() < 128, "bucket overflow"
        lrow_of[srows] = win_of * 128 + occ
    row_of = np.full((NCORES, RPAD), -1, np.int64)
    row_of[core_of, lrow_of] = np.arange(N)

    # per-token: core, window, local row m, stream, local col idx
    tcore = core_of[rows]
    tlrow = lrow_of[rows]
    twin = tlrow // 128
    tm = tlrow % 128
    tstream = (cols >= SPLIT).astype(np.int64)
    tjloc = np.where(tstream == 0, cols, cols - SPLIT)

    # counts per (core, window, stream)
    key = (tcore * nwin + twin) * 2 + tstream
    cnt = np.bincount(key, minlength=NCORES * nwin * 2).reshape(
        NCORES, nwin, 2)
    bw = -(-cnt.max(axis=0) // 128)          # [nwin, 2] blocks per stream
    BLO, BHI = bw[:, 0], bw[:, 1]
    NB = int(bw.sum())

    # tile-local block ordinals and global stream block indices
    lo_base = np.zeros(nwin + 1, np.int64)
    np.cumsum(BLO, out=lo_base[1:])
    hi_base = np.zeros(nwin + 1, np.int64)
    np.cumsum(BHI, out=hi_base[1:])
    tile_base = np.zeros(nwin + 1, np.int64)
    np.cumsum(BLO + BHI, out=tile_base[1:])
    ncalls_lo = int(-(-lo_base[-1] // gb))
    ncalls_hi = int(-(-hi_base[-1] // gb))

    tilemeta = []
    for w in range(nwin):
        meta = []
        for k in range(int(BLO[w])):
            sbi = lo_base[w] + k
            meta.append((0, int(sbi // gb), int(sbi % gb)))
        for k in range(int(BHI[w])):
            sbi = hi_base[w] + k
            meta.append((1, int(sbi // gb), int(sbi % gb)))
        tilemeta.append(meta)

    # Prefill pad slots with spread row ids: all-zero pad idxs all hit HBM
    # row 0, serializing on one bank (measured 5x slower when degenerate).
    def _spread(ncalls, lim):
        a = (np.arange(ncalls * cpw * 16, dtype=np.int64) * 97) % lim
        return np.broadcast_to(
            a.astype(np.int16).reshape(1, ncalls * cpw, 16).transpose(0, 2, 1),
            (NCORES, 16, ncalls * cpw)).copy()

    idx_lo = _spread(ncalls_lo, SPLIT)
    idx_hi = _spread(ncalls_hi, NHI)
    lrow = np.zeros((NCORES, 128, NB), np.float32)
    val = np.zeros((NCORES, 128, NB), np.float32)

    # slot assignment: stable-sort tokens by (core, window, stream)
    sort2 = np.argsort(key, kind="stable")
    ks = key[sort2]
    gstarts = np.searchsorted(ks, np.arange(NCORES * nwin * 2), side="left")
    q = np.arange(E) - gstarts[ks]           # slot within (c,w,s) group
    c2 = tcore[sort2]
    w2 = twin[sort2]
    s2 = tstream[sort2]
    m2 = tm[sort2]
    j2 = tjloc[sort2]
    v2 = vals[sort2]
    kblk = q // 128                          # block within (w, s)
    pslot = q % 128
    # tile-local block ordinal
    bo = np.where(s2 == 0, kblk, BLO[w2] + kblk) + tile_base[w2]
    lrow[c2, pslot, bo] = m2
    val[c2, pslot, bo] = v2
    # gather stream position
    sbi = np.where(s2 == 0, lo_base[w2], hi_base[w2]) + kblk
    cidx = sbi // gb
    kk = sbi % gb
    gi = kk * 128 + pslot                    # index within call
    gcol = cidx * cpw + gi // 16
    gpart = gi % 16
    idx16 = j2.astype(np.int16)
    m_lo = s2 == 0
    idx_lo[c2[m_lo], gpart[m_lo], gcol[m_lo]] = idx16[m_lo]
    m_hi = ~m_lo
    idx_hi[c2[m_hi], gpart[m_hi], gcol[m_hi]] = idx16[m_hi]
    # replicate the 16-partition index groups x8 (one copy per Q7 core)
    idx_lo = np.ascontiguousarray(np.tile(idx_lo, (1, 8, 1)))
    idx_hi = np.ascontiguousarray(np.tile(idx_hi, (1, 8, 1)))

    assert max(len(m) for m in tilemeta) <= 24, "sel tile too small"
    assert min(len(m) for m in tilemeta) >= 1
    blocks = [(int(BLO[w]), int(BHI[w])) for w in range(nwin)]
    return dict(row_of=row_of, blocks=blocks, tilemeta=tilemeta,
                ncalls=(ncalls_lo, ncalls_hi), idx_lo=idx_lo, idx_hi=idx_hi,
                lrow=lrow, val=val, NB=NB, gb=gb, cpw=cpw)


def build_spmm(sched, repeat=1, gbufs=None, selbufs=None, psbufs=None,
               skip_mm=False, skip_gather=False, skip_sel=False,
               nqueues=4, scratch=None):
    import concourse.bacc as bacc
    import concourse.tile as tile
    import concourse.mybir as mybir
    from concourse import library_config

    gb = sched["gb"]
    cpw = sched["cpw"]
    gbufs = GBUFS if gbufs is None else gbufs
    selbufs = SELBUFS if selbufs is None else selbufs
    psbufs = PSBUFS if psbufs is None else psbufs
    ncalls_lo, ncalls_hi = sched["ncalls"]
    NBt = sched["NB"]
    tilemeta = sched["tilemeta"]
    selmax = max(len(m) for m in tilemeta)
    kw = {} if scratch is None else dict(dynamic_dma_scratch_size=scratch)
    nc = bacc.Bacc("TRN2", target_bir_lowering=False, debug=False,
                   num_devices=NCORES, num_swdge_queues=nqueues, **kw)
    bf = mybir.dt.bfloat16
    tlo = nc.dram_tensor("tlo", [SPLIT, D], bf, kind="ExternalInput")
    thi = nc.dram_tensor("thi", [NHI, D], bf, kind="ExternalInput")
    idxlo = nc.dram_tensor("idxlo", [128, ncalls_lo * cpw], mybir.dt.int16,
                           kind="ExternalInput")
    idxhi = nc.dram_tensor("idxhi", [128, ncalls_hi * cpw], mybir.dt.int16,
                           kind="ExternalInput")
    lrow = nc.dram_tensor("lrow", [128, NBt], mybir.dt.float32,
                          kind="ExternalInput")
    val = nc.dram_tensor("val", [128, NBt], mybir.dt.float32,
                         kind="ExternalInput")
    iota = nc.dram_tensor("iota", [128, 128], mybir.dt.float32,
                          kind="ExternalInput")
    if SELMODE == "dma":
        seldram = nc.dram_tensor("sel", [128, NBt * 128], bf,
                                 kind="ExternalInput")
    sout = nc.dram_tensor("sout", [RPAD, D], bf, kind="ExternalOutput")
    tabs = [tlo, thi]
    idxts = []
    with tile.TileContext(nc) as tc:
        with (
            tc.tile_pool(name="const", bufs=1) as cpool,
            tc.tile_pool(name="glo", bufs=gbufs) as glopool,
            tc.tile_pool(name="ghi", bufs=gbufs) as ghipool,
            tc.tile_pool(name="selp", bufs=selbufs) as selpool,
            tc.tile_pool(name="stp", bufs=3) as stpool,
            tc.tile_pool(name="psum", bufs=psbufs, space="PSUM") as ppool,
        ):
            nc.gpsimd.load_library(library_config.mlp)
            it_lo = cpool.tile([128, ncalls_lo * cpw], mybir.dt.int16)
            nc.sync.dma_start(it_lo[:], idxlo[:])
            it_hi = cpool.tile([128, ncalls_hi * cpw], mybir.dt.int16)
            nc.sync.dma_start(it_hi[:], idxhi[:])
            lrow_t = cpool.tile([128, NBt], mybir.dt.float32)
            nc.sync.dma_start(lrow_t[:], lrow[:])
            val_t = cpool.tile([128, NBt], mybir.dt.float32)
            nc.sync.dma_start(val_t[:], val[:])
            iota_t = cpool.tile([128, 128], mybir.dt.float32)
            nc.sync.dma_start(iota_t[:], iota[:])
            idxts = [it_lo, it_hi]
            gpools = [glopool, ghipool]
            total_calls = [ncalls_lo, ncalls_hi]
            nblocks_stream = [0, 0]
            for meta in tilemeta:
                for (s, cidx, kk) in meta:
                    nblocks_stream[s] = max(nblocks_stream[s], cidx * gb + kk + 1)
            with tc.For_i(0, repeat):
                gtiles = [{}, {}]
                qctr = [0]

                def issue(s, cidx):
                    nb_in_call = min(gb, nblocks_stream[s] - cidx * gb)
                    g = gpools[s].tile([128, gb * D], mybir.dt.bfloat16,
                                       tag=f"g{s}")
                    if not skip_gather:
                        ni = nb_in_call * 128
                        gv = g[:, :nb_in_call * D].rearrange(
                            "p (k e) -> p k e", k=nb_in_call)
                        qn = (qctr[0] % nqueues if QSPLIT is None
                              else QMAP(s, qctr[0]))
                        nc.gpsimd.dma_gather(
                            gv, tabs[s][:],
                            idxts[s][:, cidx * cpw:cidx * cpw + nb_in_call * 8],
                            ni, ni, D, queue_num=qn,
                            single_packet=SP)
                    qctr[0] += 1
                    gtiles[s][cidx] = g

                bo = 0
                for w in range(PTILES):
                    meta = tilemeta[w]
                    nbw = len(meta)
                    sel_t = selpool.tile([128, selmax * 128],
                                         mybir.dt.bfloat16, tag="sel")
                    ps = ppool.tile([128, D], mybir.dt.float32)
                    if SELMODE == "dma":
                        nc.sync.dma_start(
                            sel_t[:, :nbw * 128],
                            seldram[:, bo * 128:(bo + nbw) * 128])
                    for bi, (s, cidx, kk) in enumerate(meta):
                        if cidx not in gtiles[s]:
                            issue(s, cidx)
                        if SELMODE == "dve" and not skip_sel:
                            nc.vector.tensor_scalar(
                                sel_t[:, bi * 128:(bi + 1) * 128],
                                iota_t[:],
                                lrow_t[:, bo + bi:bo + bi + 1],
                                val_t[:, bo + bi:bo + bi + 1],
                                mybir.AluOpType.is_equal,
                                mybir.AluOpType.mult)
                        if not skip_mm:
                            nc.tensor.matmul(
                                out=ps[:],
                                lhsT=sel_t[:, bi * 128:(bi + 1) * 128],
                                rhs=gtiles[s][cidx][:, kk * D:(kk + 1) * D],
                                start=(bi == 0), stop=(bi == nbw - 1))
                    st = stpool.tile([128, D], mybir.dt.bfloat16, tag="st")
                    if skip_mm:
                        nc.scalar.copy(st[:], sel_t[:, :D])
                    else:
                        nc.scalar.copy(st[:], ps[:])
                    nc.sync.dma_start(sout[w * 128:(w + 1) * 128, :], st[:])
                    bo += nbw
    nc.compile()
    return nc


def build_spmm_fused(sched, repeat=1, gbufs=None, selbufs=None, psbufs=None):
    """Hop-2 spmm with the dense stage fused in.

    Per window w: psum S2_w as in build_spmm, then
      outT_w[fo, t*128+r] = sum_k W'[t,k]^T @ {x, S1, S2}T_w[t]
    with the three rhs operands produced by on-device tensor-engine
    transposes of [128,128] slices (x/S1 window rows loaded contiguously
    from xloc/s1loc; S2 from the st bf16 copy). Drops the sout write;
    writes outw [128, PTILES*D] instead. x W0' uses W0-W2, S2 term 2*W2
    (host-folded wp, same as build_dense).
    """
    import concourse.bacc as bacc
    import concourse.tile as tile
    import concourse.mybir as mybir
    from concourse import library_config

    gb = sched["gb"]
    cpw = sched["cpw"]
    gbufs = 14 if gbufs is None else gbufs
    selbufs = SELBUFS if selbufs is None else selbufs
    psbufs = 4 if psbufs is None else psbufs
    ncalls_lo, ncalls_hi = sched["ncalls"]
    NBt = sched["NB"]
    tilemeta = sched["tilemeta"]
    selmax = max(len(m) for m in tilemeta)
    nc = bacc.Bacc("TRN2", target_bir_lowering=False, debug=False,
                   num_devices=NCORES, num_swdge_queues=4)
    bf = mybir.dt.bfloat16
    f32 = mybir.dt.float32
    tlo = nc.dram_tensor("tlo", [SPLIT, D], bf, kind="ExternalInput")
    thi = nc.dram_tensor("thi", [NHI, D], bf, kind="ExternalInput")
    idxlo = nc.dram_tensor("idxlo", [128, ncalls_lo * cpw], mybir.dt.int16,
                           kind="ExternalInput")
    idxhi = nc.dram_tensor("idxhi", [128, ncalls_hi * cpw], mybir.dt.int16,
                           kind="ExternalInput")
    lrow = nc.dram_tensor("lrow", [128, NBt], f32, kind="ExternalInput")
    val = nc.dram_tensor("val", [128, NBt], f32, kind="ExternalInput")
    iota = nc.dram_tensor("iota", [128, 128], f32, kind="ExternalInput")
    xloc = nc.dram_tensor("xloc", [RPAD, D], bf, kind="ExternalInput")
    s1loc = nc.dram_tensor("s1loc", [RPAD, D], bf, kind="ExternalInput")
    wp = nc.dram_tensor("wp", [C, T * KCH * C], bf, kind="ExternalInput")
    ident = nc.dram_tensor("ident", [128, 128], bf, kind="ExternalInput")
    outw = nc.dram_tensor("outw", [128, PTILES * D], bf,
                          kind="ExternalOutput")
    tabs = [tlo, thi]
    with tile.TileContext(nc) as tc:
        with (
            tc.tile_pool(name="const", bufs=1) as cpool,
            tc.tile_pool(name="glo", bufs=gbufs) as glopool,
            tc.tile_pool(name="ghi", bufs=gbufs) as ghipool,
            tc.tile_pool(name="selp", bufs=selbufs) as selpool,
            tc.tile_pool(name="stp", bufs=2) as stpool,
            tc.tile_pool(name="locp", bufs=2) as locpool,
            tc.tile_pool(name="xtp", bufs=2) as xtpool,
            tc.tile_pool(name="otp", bufs=2) as otpool,
            tc.tile_pool(name="psum", bufs=psbufs, space="PSUM") as ppool,
            tc.tile_pool(name="pstr", bufs=2, space="PSUM") as trpool,
            tc.tile_pool(name="psot", bufs=2, space="PSUM") as otppool,
        ):
            nc.gpsimd.load_library(library_config.mlp)
            it_lo = cpool.tile([128, ncalls_lo * cpw], mybir.dt.int16)
            nc.sync.dma_start(it_lo[:], idxlo[:])
            it_hi = cpool.tile([128, ncalls_hi * cpw], mybir.dt.int16)
            nc.sync.dma_start(it_hi[:], idxhi[:])
            lrow_t = cpool.tile([128, NBt], f32)
            nc.sync.dma_start(lrow_t[:], lrow[:])
            val_t = cpool.tile([128, NBt], f32)
            nc.sync.dma_start(val_t[:], val[:])
            iota_t = cpool.tile([128, 128], f32)
            nc.sync.dma_start(iota_t[:], iota[:])
            w_t = cpool.tile([128, T * KCH * C], bf)
            nc.sync.dma_start(w_t[:], wp[:])
            id_t = cpool.tile([128, 128], bf)
            nc.sync.dma_start(id_t[:], ident[:])
            idxts = [it_lo, it_hi]
            gpools = [glopool, ghipool]
            nblocks_stream = [0, 0]
            for meta in tilemeta:
                for (s, cidx, kk) in meta:
                    nblocks_stream[s] = max(nblocks_stream[s],
                                            cidx * gb + kk + 1)
            with tc.For_i(0, repeat):
                gtiles = [{}, {}]
                qctr = [0]

                def issue(s, cidx):
                    nb_in_call = min(gb, nblocks_stream[s] - cidx * gb)
                    g = gpools[s].tile([128, gb * D], bf, tag=f"g{s}")
                    ni = nb_in_call * 128
                    gv = g[:, :nb_in_call * D].rearrange(
                        "p (k e) -> p k e", k=nb_in_call)
                    nc.gpsimd.dma_gather(
                        gv, tabs[s][:],
                        idxts[s][:, cidx * cpw:cidx * cpw + nb_in_call * 8],
                        ni, ni, D, queue_num=QMAP(s, qctr[0]),
                        single_packet=SP)
                    qctr[0] += 1
                    gtiles[s][cidx] = g

                def fuse_dense(w, st, xw, s1w):
                    # dense stage for window w: XT slices via tensor-engine
                    # transposes, then 12 [128x128] matmuls into outT psum.
                    # Called one window late so every input is long ready
                    # when these instructions dispatch (avoids head-of-line
                    # stalls in the per-engine static order).
                    xt_all = xtpool.tile([128, 3 * D], bf, tag="xt")
                    for si, src in enumerate((xw, s1w, st)):
                        tp = trpool.tile([128, D], bf, tag="tp")
                        for t in range(T):
                            nc.tensor.transpose(
                                tp[:, t * C:(t + 1) * C],
                                src[:, t * C:(t + 1) * C], id_t[:])
                        nc.scalar.copy(
                            xt_all[:, si * D:(si + 1) * D], tp[:])
                    ot_ps = otppool.tile([128, D], f32)
                    for t in range(T):
                        for k in range(KCH):
                            nc.tensor.matmul(
                                out=ot_ps[:, t * C:(t + 1) * C],
                                lhsT=w_t[:, (t * KCH + k) * C:
                                         (t * KCH + k + 1) * C],
                                rhs=xt_all[:, k * D + t * C:
                                           k * D + (t + 1) * C],
                                start=(k == 0), stop=(k == KCH - 1))
                    ot = otpool.tile([128, D], bf, tag="ot")
                    nc.scalar.copy(ot[:], ot_ps[:])
                    nc.sync.dma_start(outw[:, w * D:(w + 1) * D], ot[:])

                bo = 0
                pend = None
                for w in range(PTILES):
                    meta = tilemeta[w]
                    nbw = len(meta)
                    sel_t = selpool.tile([128, selmax * 128], bf, tag="sel")
                    ps = ppool.tile([128, D], f32)
                    xw = locpool.tile([128, D], bf, tag="xw")
                    nc.sync.dma_start(xw[:], xloc[w * 128:(w + 1) * 128, :])
                    s1w = locpool.tile([128, D], bf, tag="s1w")
                    nc.sync.dma_start(s1w[:], s1loc[w * 128:(w + 1) * 128, :])
                    for bi, (s, cidx, kk) in enumerate(meta):
                        if cidx not in gtiles[s]:
                            issue(s, cidx)
                        nc.vector.tensor_scalar(
                            sel_t[:, bi * 128:(bi + 1) * 128],
                            iota_t[:],
                            lrow_t[:, bo + bi:bo + bi + 1],
                            val_t[:, bo + bi:bo + bi + 1],
                            mybir.AluOpType.is_equal,
                            mybir.AluOpType.mult)
                        nc.tensor.matmul(
                            out=ps[:],
                            lhsT=sel_t[:, bi * 128:(bi + 1) * 128],
                            rhs=gtiles[s][cidx][:, kk * D:(kk + 1) * D],
                            start=(bi == 0), stop=(bi == nbw - 1))
                    st = stpool.tile([128, D], bf, tag="st")
                    nc.scalar.copy(st[:], ps[:])
                    if pend is not None:
                        fuse_dense(*pend)
                    pend = (w, st, xw, s1w)
                    bo += nbw
                fuse_dense(*pend)
    nc.compile()
    return nc


def build_dense(repeat=1):
    import concourse.bacc as bacc
    import concourse.tile as tile
    import concourse.mybir as mybir

    nc = bacc.Bacc("TRN2", target_bir_lowering=False, debug=False,
                   num_devices=NCORES)
    bf = mybir.dt.bfloat16
    xT = nc.dram_tensor("xT", [D, RPAD_D], bf, kind="ExternalInput")
    s1T = nc.dram_tensor("s1T", [D, RPAD_D], bf, kind="ExternalInput")
    s2T = nc.dram_tensor("s2T", [D, RPAD_D], bf, kind="ExternalInput")
    wp = nc.dram_tensor("wp", [C, T * KCH * C], bf, kind="ExternalInput")
    outT = nc.dram_tensor("outT", [T, C, RPAD_D], bf, kind="ExternalOutput")
    srcs = [xT, s1T, s2T]
    with tile.TileContext(nc) as tc:
        with (
            tc.tile_pool(name="wpool", bufs=1) as wpool,
            tc.tile_pool(name="rhsp", bufs=2) as rhspool,
            tc.tile_pool(name="outp", bufs=2) as outpool,
            tc.tile_pool(name="psum", bufs=4, space="PSUM") as ppool,
        ):
            w_t = wpool.tile([128, T * KCH * C], bf)
            nc.sync.dma_start(w_t[:], wp[:])
            with tc.For_i(0, repeat):
                for t in range(T):
                    rhss = []
                    for k in range(KCH):
                        rhs = rhspool.tile([128, RPAD_D], bf, tag=f"rhs{k}")
                        nc.sync.dma_start(rhs[:], srcs[k][t * C:(t + 1) * C, :])
                        rhss.append(rhs)
                    ot = outpool.tile([128, RPAD_D], bf, tag="ot")
                    for dw in range(DTILES):
                        ps = ppool.tile([128, DW], mybir.dt.float32)
                        for k in range(KCH):
                            nc.tensor.matmul(
                                out=ps[:],
                                lhsT=w_t[:, (t * KCH + k) * C:(t * KCH + k + 1) * C],
                                rhs=rhss[k][:, dw * DW:(dw + 1) * DW],
                                start=(k == 0), stop=(k == KCH - 1))
                        nc.vector.tensor_copy(ot[:, dw * DW:(dw + 1) * DW], ps[:])
                    nc.sync.dma_start(outT[t, :, :], ot[:])
    nc.compile()
    return nc


_IOTA = np.tile(np.arange(128, dtype=np.float32), (128, 1))


def host_sel(sched):
    if "hostsel" in sched:
        return sched["hostsel"]
    NB = sched["NB"]
    sel = np.zeros((NCORES, 128, NB, 128), BF16)
    li = sched["lrow"].astype(np.int64)
    cc, pp, bb = np.meshgrid(np.arange(NCORES), np.arange(128),
                             np.arange(NB), indexing="ij")
    sel[cc, pp, bb, li] = sched["val"].astype(BF16)
    sched["hostsel"] = sel.reshape(NCORES, 128, NB * 128)
    return sched["hostsel"]


def _spmm_inputs(sched, tab_bf):
    """Per-core input dicts for one hop given the bf16 table [N, D]."""
    tlo = np.ascontiguousarray(tab_bf[:SPLIT])
    thi = np.ascontiguousarray(tab_bf[SPLIT:])
    ins = [
        {"tlo": tlo, "thi": thi,
         "idxlo": sched["idx_lo"][c], "idxhi": sched["idx_hi"][c],
         "lrow": sched["lrow"][c], "val": sched["val"][c], "iota": _IOTA}
        for c in range(NCORES)]
    if SELMODE == "dma":
        hs = host_sel(sched)
        for c in range(NCORES):
            ins[c]["sel"] = hs[c]
    return ins


_IDENT = np.eye(128, dtype=BF16)


def _spmm2_inputs(sched, tab_bf, xlocs, s1locs, wp):
    """Per-core input dicts for the fused hop-2 program."""
    ins = _spmm_inputs(sched, tab_bf)
    for c in range(NCORES):
        ins[c].update({"xloc": xlocs[c], "s1loc": np.asarray(s1locs[c]),
                       "wp": wp, "ident": _IDENT})
    return ins


def kernel(x, edge_row, edge_col, edge_val, weight, bias):
    from concourse.bass_utils import run_bass_kernel_spmd

    x = np.asarray(x, dtype=np.float32)
    edge_row = np.asarray(edge_row).astype(np.int64)
    edge_col = np.asarray(edge_col).astype(np.int64)
    edge_val = np.asarray(edge_val, dtype=np.float32)
    weight = np.asarray(weight, dtype=np.float32)
    bias = np.asarray(bias, dtype=np.float32)

    h = np.ascontiguousarray(x.transpose(1, 0, 2).reshape(N, D))  # [N, T*C]
    h_bf = h.astype(BF16)
    wp = np.stack(
        [weight[:, 0] - weight[:, 2], weight[:, 1], 2.0 * weight[:, 2]],
        axis=1)  # [T, 3, C, C]
    wp = np.ascontiguousarray(
        wp.transpose(2, 0, 1, 3).reshape(C, T * KCH * C)).astype(BF16)

    fp = (E, int(edge_row[:64].sum()), int(edge_col[:64].sum()),
          int(edge_row[-64:].sum()), int(edge_col[-64:].sum()))
    if _cache.get("fp") != fp:
        _cache.clear()
        _cache["fp"] = fp
        _cache["sched"] = build_schedule(edge_row, edge_col, edge_val)
    sched = _cache["sched"]
    if "spmm" not in _cache:
        _cache["spmm"] = build_spmm(sched)
    if "spmm2" not in _cache:
        _cache["spmm2"] = build_spmm_fused(sched)
    nc_s, nc_f = _cache["spmm"], _cache["spmm2"]
    row_of = sched["row_of"]
    valid = row_of >= 0
    clamped = np.maximum(row_of, 0)

    # ---- hop 1: S1 = L @ h ----
    r1 = run_bass_kernel_spmd(nc_s, _spmm_inputs(sched, h_bf),
                              core_ids=list(range(NCORES)))
    s1_parts = [r1.results[c]["sout"] for c in range(NCORES)]  # bf16 [RPAD,D]
    s1 = np.zeros((N, D), BF16)
    for c in range(NCORES):
        s1[row_of[c][valid[c]]] = s1_parts[c][valid[c]]

    # ---- hop 2 (fused): S2 = L @ S1; out = x W0' + S1 W1 + S2 2W2 ----
    xlocs = [np.where(valid[c][:, None], h[clamped[c]], 0.0).astype(BF16)
             for c in range(NCORES)]
    r2 = run_bass_kernel_spmd(
        nc_f, _spmm2_inputs(sched, s1, xlocs, s1_parts, wp),
        core_ids=list(range(NCORES)))

    out = np.empty((T, N, C), np.float32)
    for c in range(NCORES):
        ow = np.asarray(r2.results[c]["outw"], dtype=np.float32)
        loc = ow.reshape(C, PTILES, T, 128).transpose(2, 1, 3, 0).reshape(
            T, RPAD, C)
        out[:, row_of[c][valid[c]], :] = loc[:, valid[c], :]
    out += bias[:, None, :]
    return out

